# revision 1
# baseline (speedup 1.0000x reference)
"""Trainium2 Bass kernel for DiffSortNet (differentiable bitonic sort network).

Full inputs in, full outputs out. Pure data parallel over 8 NeuronCores
(batch 512 -> 64 per core). The one-hot selector matrices are compile-time
constants of the bitonic network for n=256, so the kernel derives the
(lo, hi, direction) structure itself; only `vectors` goes to the device.

Math (per batch b, layer with pair distance m):
    pairs (lo, hi=lo+m), direction flag = bit_{block+1}(lo)
    dv = (v[hi]-v[lo]) * (flag ? -1 : +1);  q = arctan(10*dv)/pi + 0.5
    X[:,lo], X[:,hi] = H + q*(L-H), L - q*(L-H)      (L/H = old X cols)

Performance structure:
  * window sparsity: after block bi, column j of X is supported only on
    rows i inside the aligned 2^(bi+1)-window of j, so each butterfly op
    only touches i inside the (window of the pair) -> ~2.4x fewer elements.
  * 4-pass in-place update via swapped writes: newLo is written AT the old
    hi position and newHi at the old lo position, so each op reads/writes
    the same columns (no WAR hazard, no 5th pass). This leaves the columns
    physically permuted by XOR mask M (M ^= m per layer); bookkeeping is
    compile-time. Four layers (one for each m with odd multiplicity:
    m=2,8,32,128) instead use the 5-pass non-swapping form so the final
    M is 0 and the output DMA is straight.
  * v_perm (the vector entries in current physical column order) is
    maintained by two small copies per swapping layer, so dv/q for layer t
    can be computed with plain strided slices.

SBUF layout: partition p = h*64 + b (h = i-half of X), free = (i_lo, j).
q broadcasts along i via a zero-stride AP dim.
"""
import math
import sys
from contextlib import ExitStack

sys.path.insert(0, "/opt/trn_rl_repo")

import numpy as np

import concourse.bacc as bacc
import concourse.bass as bass
import concourse.mybir as mybir
import concourse.tile as tile
from concourse.bass_utils import run_bass_kernel_spmd

N = 256
B_FULL = 512
N_CORES = 8
B_LOC = B_FULL // N_CORES  # 64
STEEP = 10.0
FP = mybir.dt.float32
LOG2N = 8
XFREE = 128 * N  # x tile free size (per-partition f32 elements)
FD_CAP = 2048    # max free elements per butterfly instruction (scratch cap)


def _layer_structure(n=N):
    """[(block, layer, m, flag_bit, swap)] for the 36 layers. `swap` marks
    layers using the 4-pass swapped-write form; one layer per odd-multiplicity
    m (2, 8, 32, 128) uses the 5-pass form so XOR masks cancel to 0."""
    out = []
    noswap = {(1, 0), (3, 0), (5, 0), (7, 0)}
    for bi in range(int(math.log2(n))):
        for li in range(bi + 1):
            m = 2 ** (bi - li)
            out.append((bi, li, m, bi + 1, (bi, li) not in noswap))
    return out


LAYERS = _layer_structure()
L = len(LAYERS)  # 36


def emit(tc, v_in, x_out, n_layers=L):
    nc = tc.nc
    O = mybir.AluOpType
    A = mybir.ActivationFunctionType
    with ExitStack() as ctx:
        pool = ctx.enter_context(tc.tile_pool(name="main", bufs=1))
        scratch = ctx.enter_context(tc.tile_pool(name="scr", bufs=1))
        bfly = ctx.enter_context(tc.tile_pool(name="bfly", bufs=2))

        # ---- load vectors, replicated across h (p = h*64 + b) ----
        va = pool.tile([128, N], FP, tag="va")
        vb = pool.tile([128, N], FP, tag="vb")
        nc.sync.dma_start(va[0:B_LOC, :], v_in)
        nc.sync.dma_start(va[B_LOC:128, :], v_in)

        # ---- q_all[p, t*128+k]: dv with sign fold + range-reduced arctan --
        # v_perm state: v_cur[phys col c] = v[c ^ M]; M advances on swap layers
        q_all = pool.tile([128, n_layers * 128], FP, tag="qa")
        v_cur, v_nxt = va, vb
        M = 0
        Q_CHUNK = 6
        for t0 in range(0, n_layers, Q_CHUNK):
            tn = min(Q_CHUNK, n_layers - t0)
            cw = tn * 128
            z = scratch.tile([128, Q_CHUNK * 128], FP, tag="z")
            t1 = scratch.tile([128, Q_CHUNK * 128], FP, tag="t1")
            t2 = scratch.tile([128, Q_CHUNK * 128], FP, tag="t2")
            t3 = scratch.tile([128, Q_CHUNK * 128], FP, tag="t3")
            mk = scratch.tile([128, Q_CHUNK * 128], mybir.dt.uint8, tag="mk")
            for ti in range(tn):
                t = t0 + ti
                bi, li, m, fb, swap = LAYERS[t]
                ngrp = N // (2 * m)
                lm = bi - li  # log2(m)
                Mlm = (M >> lm) & 1
                vv = v_cur[:].rearrange("p (g r j) -> p g r j", g=ngrp, r=2)
                vlo = vv[:, :, Mlm, :]      # logical-lo values  [p, g, m]
                vhi = vv[:, :, 1 - Mlm, :]  # logical-hi values
                dv_t = z[:, ti * 128 : (ti + 1) * 128].rearrange(
                    "p (g j) -> p g j", g=ngrp
                )
                if fb < LOG2N:
                    # flag = bit_fb(logical lo) = bit gbit of physical g,
                    # XOR M_fb; gbit = fb - lm - 1
                    gbit = fb - lm - 1
                    go = 2 ** gbit
                    Mfb = (M >> fb) & 1
                    ga = dv_t.rearrange("p (a f g) j -> p a f g j", f=2, g=go)
                    vl = vlo.rearrange("p (a f g) j -> p a f g j", f=2, g=go)
                    vh = vhi.rearrange("p (a f g) j -> p a f g j", f=2, g=go)
                    asc, dsc = Mfb, 1 - Mfb
                    nc.vector.tensor_tensor(
                        ga[:, :, asc], vh[:, :, asc], vl[:, :, asc], O.subtract
                    )
                    nc.vector.tensor_tensor(
                        ga[:, :, dsc], vl[:, :, dsc], vh[:, :, dsc], O.subtract
                    )
                else:
                    nc.vector.tensor_tensor(dv_t, vhi, vlo, O.subtract)
                if swap:
                    # v_nxt[c] = v_cur[c ^ m]
                    vn = v_nxt[:].rearrange("p (g r j) -> p g r j", g=ngrp, r=2)
                    nc.scalar.copy(vn[:, :, 0, :], vv[:, :, 1, :])
                    nc.scalar.copy(vn[:, :, 1, :], vv[:, :, 0, :])
                    v_cur, v_nxt = v_nxt, v_cur
                    M ^= m

            # q = arctan(10*z)/pi + 0.5, range-reduced:
            #   |10z| < 1 : arctan(10z);  else sign(z)*(pi/2 - arctan(1/|10z|))
            # 1/w via exp(-ln(max(w,1))) (ACT Reciprocal banned, DVE recip slow)
            qc = q_all[:, t0 * 128 : t0 * 128 + cw]
            zc, t1c, t2c, t3c = z[:, :cw], t1[:, :cw], t2[:, :cw], t3[:, :cw]
            nc.vector.tensor_scalar(t1c, zc, -0.1, 0.1, O.max, O.min)
            nc.scalar.activation(qc, t1c, A.Arctan, bias=0.0, scale=STEEP)
            nc.scalar.activation(t1c, zc, A.Abs, bias=0.0, scale=STEEP)
            nc.gpsimd.tensor_scalar(t2c, t1c, 1.0, None, O.max)
            nc.scalar.activation(t2c, t2c, A.Ln)
            nc.scalar.activation(t2c, t2c, A.Exp, bias=0.0, scale=-1.0)
            nc.scalar.activation(t2c, t2c, A.Arctan)
            nc.scalar.activation(t3c, zc, A.Sign)
            nc.gpsimd.tensor_tensor(t2c, t3c, t2c, O.mult)
            nc.vector.scalar_tensor_tensor(
                t2c, t3c, float(math.pi / 2), t2c, O.mult, O.subtract
            )
            nc.gpsimd.tensor_scalar(mk[:, :cw], t1c, 1.0, None, O.is_lt)
            nc.vector.copy_predicated(t2c, mk[:, :cw], qc)
            nc.scalar.activation(qc, t2c, A.Copy,
                                 bias=0.5, scale=float(1.0 / math.pi))

        # ---- init X = identity ----
        x = pool.tile([128, XFREE], FP, tag="x")
        nc.vector.memset(x[:], 0.0)
        x3 = x[:].rearrange("p (i j) -> p i j", j=N)
        xh = x3.rearrange("(h b) i j -> h b i j", h=2)
        diag0 = xh[0].rearrange("b i j -> b (i j)")[:, :: N + 1]
        diag1 = xh[1].rearrange("b i j -> b (i j)")[:, 128 :: N + 1][:, :128]
        nc.vector.memset(diag0, 1.0)
        nc.vector.memset(diag1, 1.0)

        xt = x[:].tensor  # handle for raw APs
        NL = n_layers * 128
        qt = q_all[:].tensor

        def _mk(tensor, off, pdim, dims):
            dims = [d for d in dims if d[1] != 1]  # HW ISA: <=3 free dims
            assert len(dims) <= 3, dims
            return bass.AP(tensor, off, [pdim] + dims)

        def x_ap(h, w, role, m, d0, ce, W0, nw_op):
            """X operand AP: windows [W0, W0+nw_op) (diagonal i/j step),
            delta chunk [d0, d0+ce), role = bit_lm column of each pair."""
            if w <= 128:
                # W0 is h-relative: i base = W0*w, col base = W0*w + 128h
                off = ((64 * h) * XFREE + (W0 * w) * N + W0 * w + 128 * h
                       + d0 * N + role * m)
                dims = [[w * N + w, nw_op], [N, ce],
                        [2 * m, w // (2 * m)], [1, m]]
                return _mk(xt, off, [XFREE, 64], dims)
            off = d0 * N + role * m
            dims = [[N, ce], [2 * m, N // (2 * m)], [1, m]]
            return _mk(xt, off, [XFREE, 128], dims)

        def s_ap(tile_h, h, w, m, ce, nw_op):
            """Scratch AP (packed), partition-sliced to match x_ap."""
            st = tile_h[:].tensor
            if w <= 128:
                dims = [[ce * (w // 2), nw_op], [w // 2, ce],
                        [m, w // (2 * m)], [1, m]]
                return _mk(st, (64 * h) * FD_CAP, [FD_CAP, 64], dims)
            dims = [[128, ce], [m, N // (2 * m)], [1, m]]
            return _mk(st, 0, [FD_CAP, 128], dims)

        def q_ap(h, w, m, ce, t, W0, nw_op):
            if w <= 128:
                # h=1 covers the second half of the pair index range
                off = (64 * h) * NL + t * 128 + h * 64 + W0 * (w // 2)
                dims = [[w // 2, nw_op], [0, ce], [m, w // (2 * m)], [1, m]]
                return _mk(qt, off, [NL, 64], dims)
            dims = [[0, ce], [m, N // (2 * m)], [1, m]]
            return _mk(qt, t * 128, [NL, 128], dims)

        # ---- butterfly layers ----
        M = 0
        for t in range(n_layers):
            bi, li, m, fb, swap = LAYERS[t]
            w = 2 ** (bi + 1)
            lm = bi - li
            Mlm = (M >> lm) & 1
            if w <= 128:
                hs = (0, 1)
                nw = 128 // w
                dext = w            # delta extent per window
            else:
                hs = (None,)
                nw = 1
                dext = 128
            # HW ISA allows 3 free dims; loop windows if all 4 nontrivial
            nontriv = sum(1 for c in (nw, dext, w // (2 * m), m) if c > 1)
            nw_op = 1 if (nontriv > 3) else nw
            n_wops = nw // nw_op
            perchunk = nw_op * (w // 2)  # FD per delta row
            ce_max = max(1, FD_CAP // perchunk)
            for h in hs:
                hh = h if h is not None else 0
                for wi in range(n_wops):
                    W0 = wi * nw_op
                    d0 = 0
                    while d0 < dext:
                        ce = min(ce_max, dext - d0)
                        sL = x_ap(hh, w, Mlm, m, d0, ce, W0, nw_op)
                        sH = x_ap(hh, w, 1 - Mlm, m, d0, ce, W0, nw_op)
                        qb = q_ap(hh, w, m, ce, t, W0, nw_op)
                        d = bfly.tile([128, FD_CAP], FP, tag="d")
                        qd = bfly.tile([128, FD_CAP], FP, tag="qd")
                        dp = s_ap(d, hh, w, m, ce, nw_op)
                        qdp = s_ap(qd, hh, w, m, ce, nw_op)
                        nc.vector.tensor_tensor(dp, sL, sH, O.subtract)
                        nc.vector.tensor_tensor(qdp, dp, qb, O.mult)
                        if swap:
                            # newLo at H slot, newHi at L slot (no WAR hazard)
                            nc.vector.tensor_tensor(sH, sH, qdp, O.add)
                            nc.vector.tensor_tensor(sL, sL, qdp, O.subtract)
                        else:
                            d2 = scratch.tile([128, FD_CAP], FP, tag="d2")
                            d2p = s_ap(d2, hh, w, m, ce, nw_op)
                            nc.vector.tensor_tensor(d2p, dp, qdp, O.subtract)
                            nc.vector.tensor_tensor(sL, sH, qdp, O.add)
                            nc.vector.tensor_tensor(sH, sH, d2p, O.add)
                        d0 += ce
            if swap:
                M ^= m
        assert n_layers < L or M == 0, f"final XOR mask {M} != 0"

        # ---- write out (one DMA per h half) ----
        oh = x_out.rearrange("b (h i) j -> h b (i j)", h=2)
        nc.sync.dma_start(oh[0], x[0:B_LOC, :])
        nc.sync.dma_start(oh[1], x[B_LOC:128, :])


def build_nc(n_layers=L):
    nc = bacc.Bacc("TRN2", target_bir_lowering=False, debug=False)
    v_in = nc.declare_dram_parameter("vectors", [B_LOC, N], FP, isOutput=False)
    x_out = nc.declare_dram_parameter("out", [B_LOC, N, N], FP, isOutput=True)
    with tile.TileContext(nc) as tc:
        emit(tc, v_in[:], x_out[:], n_layers=n_layers)
    nc.finalize()
    return nc


_NC_CACHE = {}


def kernel(**inputs) -> np.ndarray:
    vectors = np.asarray(inputs["vectors"], dtype=np.float32)
    assert vectors.shape == (B_FULL, N)
    if "default" not in _NC_CACHE:
        _NC_CACHE["default"] = build_nc()
    nc = _NC_CACHE["default"]
    in_maps = [
        {"vectors": vectors[c * B_LOC : (c + 1) * B_LOC]} for c in range(N_CORES)
    ]
    res = run_bass_kernel_spmd(nc, in_maps, core_ids=list(range(N_CORES)))
    out = np.concatenate([res.results[c]["out"] for c in range(N_CORES)], axis=0)
    return out


if __name__ == "__main__":
    rng = np.random.default_rng(0)
    v = rng.normal(size=(B_FULL, N)).astype(np.float32)
    o = kernel(vectors=v)
    print("kernel output shape:", o.shape, o.dtype)



# revision 6
# speedup vs baseline: 3.1377x; 3.1377x over previous
"""Trainium2 Bass kernel for DiffSortNet (differentiable bitonic sort network).

Full inputs in, full outputs out; pure data parallel over 8 NeuronCores
(batch 512 -> 64 per core). Selector matrices are compile-time constants of
the bitonic network (n=256); only `vectors` is sent to the device.

v2 design (see bfly_plan.py for the shared pass plan + layout docs):
  * X kept in SBUF in fp16 -> DVE tensor_tensor runs in 2x_1P mode.
  * Uniform 128-partition instructions: upper-half rows store columns
    XOR-128, and the swap-write butterfly is role-symmetric.
  * q = arctan(10*dv)/pi + 0.5 computed on the ACT engine directly
    (single Arctan op; spline is accurate over the full argument range).
  * First-of-block layers: operand supports are disjoint -> 4 half-window
    mults (half the work). Layer 0 degenerates to a direct strided init
    (no memset needed anywhere).
  * m=1 layers (w>=128): d_alt[c] = x[c^1]-x[c] via negative-stride AP
    keeps everything contiguous (2x mode) with a single update pass.
  * NOSWAP at {(1,0),(3,0),(5,0),(7,0)} (free in the 4-mult form) makes
    the final XOR mask 0: output convert + DMA are fully contiguous.
  * Final layer is chunked over rows; fp16->fp32 convert (ACT) and the
    output DMA pipeline behind the DVE butterflies.
"""
import sys

sys.path.insert(0, "/opt/trn_rl_repo")

import numpy as np

import concourse.bacc as bacc
import concourse.bass as bass
import concourse.mybir as mybir
import concourse.tile as tile
from contextlib import ExitStack
from concourse.bass_utils import run_bass_kernel_spmd

import bfly_plan as P

N = 256
B_FULL = 512
N_CORES = 8
B_LOC = B_FULL // N_CORES
FP32 = mybir.dt.float32
FP16 = mybir.dt.float16

ALU = {"subtract": mybir.AluOpType.subtract, "add": mybir.AluOpType.add,
       "mult": mybir.AluOpType.mult}
FUNC = {"Copy": mybir.ActivationFunctionType.Copy,
        "Arctan": mybir.ActivationFunctionType.Arctan}
BUF_DT = {"x": FP16, "vc0": FP32, "vc1": FP32, "dv32": FP16, "atan32": FP16,
          "q16": FP16, "omq16": FP16, "qx16": FP16, "d16": FP16, "qd16": FP16}
BUF_F = {"x": P.XF, "vc0": 256, "vc1": 256, "dv32": P.QF, "atan32": P.QF,
         "q16": P.QF, "omq16": P.QF, "qx16": 256, "d16": P.SCR, "qd16": P.SCR}

FINAL_CE = 8  # rows per output pipeline chunk


def emit(tc, v_in, x_out):
    nc = tc.nc
    with ExitStack() as ctx:
        pool = ctx.enter_context(tc.tile_pool(name="main", bufs=1))
        scr = ctx.enter_context(tc.tile_pool(name="scr", bufs=2))
        stg = ctx.enter_context(tc.tile_pool(name="stg", bufs=2))

        tiles = {}
        for name in ("x", "vc0", "vc1", "dv32", "atan32", "q16", "omq16", "qx16"):
            tiles[name] = pool.tile([128, BUF_F[name]], BUF_DT[name], name=name, tag=name)

        # ---- v load: lower half direct, upper half column-halves swapped ----
        vc0 = tiles["vc0"]
        nc.sync.dma_start(vc0[0:64, :], v_in)
        nc.sync.dma_start(vc0[64:128, 0:128], v_in[:, 128:256])
        nc.sync.dma_start(vc0[64:128, 128:256], v_in[:, 0:128])

        def ap(spec, ps, pn):
            buf, off, dims = spec
            t = tiles[buf][:].tensor
            F = BUF_F[buf]
            dims = [d for d in dims if d[1] != 1]
            assert len(dims) <= 3, (buf, dims)
            return bass.AP(t, ps * F + off, [[F, pn]] + dims)

        def emit_op(op):
            ps, pn = op["ps"], op["pn"]
            if op["kind"] == "tt" and op["newchunk"]:
                # rotate scratch tiles for pipelining
                tiles["d16"] = scr.tile([128, P.SCR], FP16, name="d16", tag="d16")
                tiles["qd16"] = scr.tile([128, P.SCR], FP16, name="qd16", tag="qd16")
            if op["kind"] == "tt":
                eng = nc.vector if op["engine"] == "v" else nc.gpsimd
                eng.tensor_tensor(ap(op["out"], ps, pn), ap(op["in0"], ps, pn),
                                  ap(op["in1"], ps, pn), ALU[op["alu"]])
            elif op["kind"] == "copy":
                nc.vector.tensor_copy(ap(op["out"], ps, pn), ap(op["in0"], ps, pn))
            elif op["kind"] == "act":
                nc.scalar.activation(ap(op["out"], ps, pn), ap(op["in0"], ps, pn),
                                     FUNC[op["func"]], bias=op["bias"],
                                     scale=op["scale"])
            else:
                raise ValueError(op["kind"])

        # ---- q phase + init + layers 1..34 ----
        for op in P.q_phase_ops() + P.init_ops():
            emit_op(op)
        for L in P.LAYERS[1:35]:
            for op in P.bfly_layer_ops(L):
                emit_op(op)

        # ---- final layer (t=35) chunked + convert + DMA pipeline ----
        L35 = P.LAYERS[35]
        oh = x_out.rearrange("b (h i) j -> h b i j", h=2)
        for r0 in range(0, 128, FINAL_CE):
            for op in P.bfly_layer_ops(L35, rng=(r0, FINAL_CE)):
                emit_op(op)
            stage = stg.tile([128, FINAL_CE * 256], FP32, tag="stage")
            xt = tiles["x"][:].tensor
            # logical col J = c ^ 128h: h=1 converts with column-halves swapped
            src0 = bass.AP(xt, r0 * 256, [[P.XF, 64], [1, FINAL_CE * 256]])
            src1 = bass.AP(xt, 64 * P.XF + r0 * 256 + 128,
                           [[P.XF, 64], [256, FINAL_CE], [-128, 2], [1, 128]])
            st_t = stage[:].tensor
            dst0 = bass.AP(st_t, 0, [[FINAL_CE * 256, 64], [1, FINAL_CE * 256]])
            dst1 = bass.AP(st_t, 64 * FINAL_CE * 256,
                           [[FINAL_CE * 256, 64], [1, FINAL_CE * 256]])
            nc.scalar.activation(dst0, src0, FUNC["Copy"], bias=0.0, scale=1.0)
            nc.scalar.activation(dst1, src1, FUNC["Copy"], bias=0.0, scale=1.0)
            st = stage[:].tensor
            sv = lambda h: bass.AP(st, h * 64 * (FINAL_CE * 256),
                                   [[FINAL_CE * 256, 64], [1, FINAL_CE * 256]])
            for h in (0, 1):
                dst = oh[h, :, r0:r0 + FINAL_CE, :].rearrange("b i j -> b (i j)")
                nc.sync.dma_start(dst, sv(h))


def build_nc():
    nc = bacc.Bacc("TRN2", target_bir_lowering=False, debug=False)
    v_in = nc.declare_dram_parameter("vectors", [B_LOC, N], FP32, isOutput=False)
    x_out = nc.declare_dram_parameter("out", [B_LOC, N, N], FP32, isOutput=True)
    with tile.TileContext(nc) as tc:
        emit(tc, v_in[:], x_out[:])
    nc.finalize()
    return nc


_NC_CACHE = {}


def kernel(**inputs) -> np.ndarray:
    vectors = np.asarray(inputs["vectors"], dtype=np.float32)
    assert vectors.shape == (B_FULL, N)
    if "default" not in _NC_CACHE:
        _NC_CACHE["default"] = build_nc()
    nc = _NC_CACHE["default"]
    in_maps = [
        {"vectors": vectors[c * B_LOC:(c + 1) * B_LOC]} for c in range(N_CORES)
    ]
    res = run_bass_kernel_spmd(nc, in_maps, core_ids=list(range(N_CORES)))
    out = np.concatenate([res.results[c]["out"] for c in range(N_CORES)], axis=0)
    return out


if __name__ == "__main__":
    rng = np.random.default_rng(0)
    v = rng.normal(size=(B_FULL, N)).astype(np.float32)
    o = kernel(vectors=v)
    print("kernel output shape:", o.shape, o.dtype)
